# revision 4
# baseline (speedup 1.0000x reference)
"""Trainium2 Bass kernel for nn_BlockSparseMoE (top-2 of 8 experts, SwiGLU).

Strategy (8-way tensor-parallel over FFN):
  - Host: compute router (gate matmul + softmax + top-2 + renorm) in fp64,
    gather each expert's tokens into a contiguous column range of one
    shared xT matrix.
  - Device (SPMD x8): every core holds a 512-wide F-slice of ALL 8
    experts' w1/w3/w2 (same ~25 MB weight traffic as one full expert in
    the expert-parallel layout) and runs all 8192 token-expert pairs
    against its slice — exactly T*K/8 = 1024 pair-equivalents per core
    regardless of routing imbalance. Partial y outputs (transposed,
    unscaled) stream back.
  - Host: sum the 8 partial outputs, scale by the renormalized top-2
    weight, scatter-add per token.

Per-core layout:
  phase A: hT[f, t] = silu(x@w1)^T * (x@w3)^T per expert (FT=4 f-tiles of
           128), lhsT = w1 d-chunk [128, 128f], rhs = xT d-chunk
           [128, tchunk] — weights stationary, tokens moving.
  phase B: yT[d, t] = w2_slice^T @ hT, lhsT = w2 f-tile [128f, 128d],
           rhs = hT f-tile [128, tchunk] — tokens moving, so ragged
           expert tails cost no extra PE cycles; no on-device scaling.
"""

import numpy as np
import ml_dtypes

HIDDEN = 1024
FFN = 4096
NUM_EXPERTS = 8
TOP_K = 2
N_CORES = 8
FS = FFN // N_CORES          # 512-wide F-slice per core
DC = HIDDEN // 128           # 8 contraction chunks for x@w1
FT = FS // 128               # 4 f-tiles per expert slice
DT = HIDDEN // 128           # 8 output d-tiles

_BF16 = ml_dtypes.bfloat16
_nc_cache = {}


# ---------------------------------------------------------------- router ----
def _route(x, gate_w, gate_b):
    """Top-2 routing. Returns per-expert (token_idx, renorm_weight)."""
    logits = x.astype(np.float64) @ gate_w.astype(np.float64) + gate_b.astype(
        np.float64
    )
    logits -= logits.max(axis=-1, keepdims=True)
    p = np.exp(logits)
    p /= p.sum(axis=-1, keepdims=True)
    # top-2 by prob, ties broken by lower index (matches jax.lax.top_k)
    top2 = np.argsort(-p, axis=-1, kind="stable")[:, :TOP_K]
    pt = np.take_along_axis(p, top2, axis=-1)
    wt = pt / pt.sum(axis=-1, keepdims=True)
    idxs, wts = [], []
    for e in range(NUM_EXPERTS):
        mask = top2 == e  # [T, 2]
        tok = np.nonzero(mask.any(axis=-1))[0]
        w = wt[tok, np.argmax(mask[tok], axis=-1)]
        idxs.append(tok)
        wts.append(w.astype(np.float32))
    return idxs, wts


def _chunks_for(load):
    """Split a token count into moving-dim chunks: all but the last are
    multiples of 128 in [256, 512]; keep the ragged tail >= 240 when
    possible (short moving dims go LDWEIGHTS-bound)."""
    C = load
    n = max(1, -(-C // 512))
    chunks = []
    rem = C
    for i in range(n - 1):
        c = min(512, -(-rem // ((n - i) * 128)) * 128)
        chunks.append(c)
        rem -= c
    while n > 1 and rem < 240 and chunks:
        for i in range(len(chunks)):
            if rem >= 240:
                break
            if chunks[i] > 256:
                chunks[i] -= 128
                rem += 128
        else:
            break
    chunks.append(rem)
    assert sum(chunks) == C and all(c > 0 for c in chunks)
    return tuple(chunks)


def _plan(loads):
    """Per-expert chunk tuples + 128-aligned xT column offsets."""
    chunks_e, offs = [], []
    off = 0
    for l in loads:
        chunks_e.append(_chunks_for(l))
        offs.append(off)
        off += -(-l // 128) * 128
    return tuple(chunks_e), tuple(offs), off


# ------------------------------------------------------------- device IR ----
def _build(plan):
    """Per-core Bacc graph. plan = (chunks_e, offs, XWT)."""
    import concourse.bacc as bacc
    import concourse.bass as bass
    import concourse.mybir as mybir
    import concourse.tile as tile

    chunks_e, offs, XWT = plan
    XW_e = [-(-sum(ch) // 128) * 128 for ch in chunks_e]

    bf16 = mybir.dt.bfloat16
    f32 = mybir.dt.float32

    nc = bacc.Bacc("TRN2", target_bir_lowering=False, debug=False,
                   num_devices=N_CORES)

    xT_d = nc.dram_tensor("xT", [HIDDEN, XWT], bf16, kind="ExternalInput")
    # w1s/w3s host-pre-tiled per expert as [e, p, dc, FS]; w2s as
    # [e, p, ft, HIDDEN] so every DMA line is fully contiguous
    w1_d = nc.dram_tensor("w1s", [NUM_EXPERTS, 128, DC, FS], bf16,
                          kind="ExternalInput")
    w3_d = nc.dram_tensor("w3s", [NUM_EXPERTS, 128, DC, FS], bf16,
                          kind="ExternalInput")
    w2_d = nc.dram_tensor("w2s", [NUM_EXPERTS, 128, FT, HIDDEN], bf16,
                          kind="ExternalInput")
    y_d = nc.dram_tensor("y", [HIDDEN, XWT], f32, kind="ExternalOutput")

    xT_v = xT_d.ap().rearrange("(dc p) c -> p dc c", p=128)
    y_v = y_d.ap().rearrange("(dt p) c -> dt p c", p=128)

    with tile.TileContext(nc) as tc:
        with (
            tc.tile_pool(name="xe", bufs=2) as xep,
            tc.tile_pool(name="w13", bufs=2) as w13,
            tc.tile_pool(name="w2p", bufs=2) as w2p,
            tc.tile_pool(name="hp", bufs=2) as hp,
            tc.tile_pool(name="sil", bufs=4) as silp,
            tc.tile_pool(name="yo", bufs=4) as yop,
            tc.tile_pool(name="ps", bufs=2, space=bass.MemorySpace.PSUM) as ps,
            tc.tile_pool(name="yps", bufs=4, space=bass.MemorySpace.PSUM) as yps,
        ):
            # HAM pre-warm: keep the PE's activity monitor busy during the
            # DMA-bound first ~10us so real matmuls start at full clock.
            warm_sb = silp.tile([128, 128], bf16, tag="warm_in", bufs=1)
            nc.gpsimd.memset(warm_sb[:], 0.0)
            warm_ps = ps.tile([128, 128], f32, tag="ph1", name="warm_ps")
            N_WARM = 72
            for i in range(N_WARM):
                nc.tensor.matmul(warm_ps[:], warm_sb[:], warm_sb[:],
                                 start=(i == 0), stop=(i == N_WARM - 1))

            xe_tiles = {}
            w13_tiles = {}
            w2_tiles = {}

            def load_w13(e):
                w1_sb = w13.tile([128, DC, FS], bf16, tag="w1", name="w1_sb")
                w3_sb = w13.tile([128, DC, FS], bf16, tag="w3", name="w3_sb")
                nc.sync.dma_start(w1_sb[:], w1_d.ap()[e])
                nc.sync.dma_start(w3_sb[:], w3_d.ap()[e])
                w13_tiles[e] = ([(w1_sb, 0)], [(w3_sb, 0)])

            def load_xe(e, col0=0):
                if e not in xe_tiles:
                    xe_tiles[e] = xep.tile([128, DC, XW_e[e]], bf16,
                                           tag="xT", name="xe_sb")
                nc.sync.dma_start(
                    xe_tiles[e][:, :, col0:XW_e[e]],
                    xT_v[:, :, offs[e] + col0:offs[e] + XW_e[e]],
                )

            def load_w2(e):
                w2_sb = w2p.tile([128, FT, HIDDEN], bf16, tag="w2",
                                 name="w2_sb")
                nc.sync.dma_start(w2_sb[:], w2_d.ap()[e])
                w2_tiles[e] = w2_sb

            def _wslice(parts, dc):
                for tile_, base in parts:
                    if base <= dc < base + tile_.shape[1]:
                        return tile_[:, dc - base, :]
                raise AssertionError(dc)

            # startup: the first f-tile chain needs w1 dc0-3 + xT chunk0;
            # interleave those DMAs ahead of everything else.
            c0 = chunks_e[0][0]
            w1a = w13.tile([128, 4, FS], bf16, tag="w1a", bufs=1)
            w3a = w13.tile([128, 4, FS], bf16, tag="w3a", bufs=1)
            w1b = w13.tile([128, 4, FS], bf16, tag="w1b", bufs=1)
            w3b = w13.tile([128, 4, FS], bf16, tag="w3b", bufs=1)
            xe_tiles[0] = xep.tile([128, DC, XW_e[0]], bf16, tag="xT",
                                   name="xe_sb0")
            nc.sync.dma_start(w1a[:], w1_d.ap()[0][:, 0:4, :])
            nc.sync.dma_start(xe_tiles[0][:, :, 0:c0], xT_v[:, :, 0:c0])
            nc.sync.dma_start(w1b[:], w1_d.ap()[0][:, 4:DC, :])
            nc.sync.dma_start(w3a[:], w3_d.ap()[0][:, 0:4, :])
            nc.sync.dma_start(w3b[:], w3_d.ap()[0][:, 4:DC, :])
            w13_tiles[0] = ([(w1a, 0), (w1b, 4)], [(w3a, 0), (w3b, 4)])

            for e in range(NUM_EXPERTS):
                w1_parts, w3_parts = w13_tiles[e]
                t0 = 0
                for ci, chunk in enumerate(chunks_e[e]):
                    xe = xe_tiles[e]
                    hT = hp.tile([128, FT, chunk], bf16, tag="hT")
                    # ---- phase A ----
                    for ft in range(FT):
                        # prefetches ride behind the first chunk's compute
                        if ci == 0:
                            if e == 0:
                                if ft == 1:
                                    load_xe(0, col0=c0)
                                    load_w2(0)
                                elif ft == 2:
                                    load_w13(1)
                                elif ft == 3:
                                    load_xe(1)
                                    load_w2(1)
                            elif e + 1 < NUM_EXPERTS:
                                if ft == 2:
                                    load_w13(e + 1)
                                elif ft == 3:
                                    load_xe(e + 1)
                                    load_w2(e + 1)
                        ph1 = ps.tile([128, chunk], f32, tag="ph1")
                        ph3 = ps.tile([128, chunk], f32, tag="ph3")
                        for dc in range(DC):
                            nc.tensor.matmul(
                                ph1[:],
                                _wslice(w1_parts, dc)[
                                    :, ft * 128:(ft + 1) * 128],
                                xe[:, dc, t0:t0 + chunk],
                                start=(dc == 0), stop=(dc == DC - 1),
                            )
                        for dc in range(DC):
                            nc.tensor.matmul(
                                ph3[:],
                                _wslice(w3_parts, dc)[
                                    :, ft * 128:(ft + 1) * 128],
                                xe[:, dc, t0:t0 + chunk],
                                start=(dc == 0), stop=(dc == DC - 1),
                            )
                        sil = silp.tile([128, chunk], bf16, tag="sil")
                        nc.scalar.activation(
                            sil[:], ph1[:], mybir.ActivationFunctionType.Silu
                        )
                        nc.vector.tensor_mul(hT[:, ft, :], sil[:], ph3[:])

                    # ---- phase B: yT[d, t] partial, unscaled ----
                    w2_sb = w2_tiles[e]
                    for dt in range(DT):
                        yp = yps.tile([128, chunk], f32, tag="yp")
                        for ft in range(FT):
                            nc.tensor.matmul(
                                yp[:],
                                w2_sb[:, ft, dt * 128:(dt + 1) * 128],
                                hT[:, ft, :],
                                start=(ft == 0), stop=(ft == FT - 1),
                            )
                        ysb = yop.tile([128, chunk], f32, tag="ysb")
                        # alternate copies between ScalarE and DVE so
                        # neither engine becomes the bottleneck
                        if dt % 2 == 0:
                            nc.scalar.copy(ysb[:], yp[:])
                        else:
                            nc.vector.tensor_copy(ysb[:], yp[:])
                        nc.sync.dma_start(
                            y_v[dt][:, offs[e] + t0:offs[e] + t0 + chunk],
                            ysb[:],
                        )
                    t0 += chunk
    nc.compile()
    return nc


def _get_nc(plan):
    if plan not in _nc_cache:
        _nc_cache[plan] = _build(plan)
    return _nc_cache[plan]


# ---------------------------------------------------------------- kernel ----
def kernel(hidden_states, gate_w, gate_b, w1, w3, w2, _trace=False):
    from concourse.bass_utils import run_bass_kernel_spmd

    B, S, D = hidden_states.shape
    T = B * S
    x = np.asarray(hidden_states, np.float32).reshape(T, D)
    idxs, wts = _route(x, np.asarray(gate_w, np.float32),
                       np.asarray(gate_b, np.float32))
    loads = [len(i) for i in idxs]
    plan = _plan(loads)
    chunks_e, offs, XWT = plan
    nc = _get_nc(plan)

    # shared xT: every expert's tokens in its 128-aligned column range
    xT = np.zeros((D, XWT), _BF16)
    for e in range(NUM_EXPERTS):
        xT[:, offs[e]:offs[e] + loads[e]] = x[idxs[e]].T.astype(_BF16)

    # weights pre-tiled for all cores in one reshape/transpose:
    # w1/w3 [E, D, F] -> [core, E, 128, dc, FS]
    w1 = np.asarray(w1, np.float32)
    w3 = np.asarray(w3, np.float32)
    w2 = np.asarray(w2, np.float32)
    w1t = np.ascontiguousarray(
        w1.reshape(NUM_EXPERTS, DC, 128, N_CORES, FS)
        .transpose(3, 0, 2, 1, 4)).astype(_BF16)
    w3t = np.ascontiguousarray(
        w3.reshape(NUM_EXPERTS, DC, 128, N_CORES, FS)
        .transpose(3, 0, 2, 1, 4)).astype(_BF16)
    # w2 [E, F, D] -> [core, E, 128, ft, D]
    w2t = np.ascontiguousarray(
        w2.reshape(NUM_EXPERTS, N_CORES, FT, 128, HIDDEN)
        .transpose(1, 0, 3, 2, 4)).astype(_BF16)

    in_maps = [{
        "xT": xT,
        "w1s": w1t[c],
        "w3s": w3t[c],
        "w2s": w2t[c],
    } for c in range(N_CORES)]

    res = run_bass_kernel_spmd(nc, in_maps, core_ids=list(range(N_CORES)),
                               trace=_trace)

    yT = res.results[0]["y"].astype(np.float32).copy()
    for c in range(1, N_CORES):
        yT += res.results[c]["y"]
    out = np.zeros((T, D), np.float32)
    for e in range(NUM_EXPERTS):
        tok, wt = idxs[e], wts[e]
        seg = yT[:, offs[e]:offs[e] + loads[e]].T  # [load, D]
        out[tok] += wt[:, None] * seg
    out = out.reshape(B, S, D)
    if _trace:
        return out, res
    return out


# revision 17
# speedup vs baseline: 1.1875x; 1.1875x over previous
"""Trainium2 Bass kernel for nn_BlockSparseMoE (top-2 of 8 experts, SwiGLU).

Strategy (8-way tensor-parallel over FFN):
  - Host: compute router (gate matmul + softmax + top-2 + renorm) in fp64,
    gather each expert's tokens into a contiguous column range of one
    shared xT matrix.
  - Device (SPMD x8): every core holds a 512-wide F-slice of ALL 8
    experts' w1/w3/w2 (same ~25 MB weight traffic as one full expert in
    the expert-parallel layout) and runs all 8192 token-expert pairs
    against its slice — exactly T*K/8 = 1024 pair-equivalents per core
    regardless of routing imbalance. Partial y outputs (transposed,
    unscaled) stream back.
  - Host: sum the 8 partial outputs, scale by the renormalized top-2
    weight, scatter-add per token.

Per-core layout:
  phase A: hT[f, t] = silu(x@w1)^T * (x@w3)^T per expert (FT=4 f-tiles of
           128), lhsT = w1 d-chunk [128, 128f], rhs = xT d-chunk
           [128, tchunk] — weights stationary, tokens moving.
  phase B: yT[d, t] = w2_slice^T @ hT, lhsT = w2 f-tile [128f, 128d],
           rhs = hT f-tile [128, tchunk] — tokens moving, so ragged
           expert tails cost no extra PE cycles; no on-device scaling.
"""

import numpy as np
import ml_dtypes

HIDDEN = 1024
FFN = 4096
NUM_EXPERTS = 8
TOP_K = 2
N_CORES = 8
FS = FFN // N_CORES          # 512-wide F-slice per core
DC = HIDDEN // 128           # 8 contraction chunks for x@w1
FT = FS // 128               # 4 f-tiles per expert slice
DT = HIDDEN // 128           # 8 output d-tiles

_BF16 = ml_dtypes.bfloat16
_nc_cache = {}


# ---------------------------------------------------------------- router ----
def _route(x, gate_w, gate_b):
    """Top-2 routing. Returns per-expert (token_idx, renorm_weight)."""
    logits = x.astype(np.float64) @ gate_w.astype(np.float64) + gate_b.astype(
        np.float64
    )
    logits -= logits.max(axis=-1, keepdims=True)
    p = np.exp(logits)
    p /= p.sum(axis=-1, keepdims=True)
    # top-2 by prob, ties broken by lower index (matches jax.lax.top_k)
    top2 = np.argsort(-p, axis=-1, kind="stable")[:, :TOP_K]
    pt = np.take_along_axis(p, top2, axis=-1)
    wt = pt / pt.sum(axis=-1, keepdims=True)
    idxs, wts = [], []
    for e in range(NUM_EXPERTS):
        mask = top2 == e  # [T, 2]
        tok = np.nonzero(mask.any(axis=-1))[0]
        w = wt[tok, np.argmax(mask[tok], axis=-1)]
        idxs.append(tok)
        wts.append(w.astype(np.float32))
    return idxs, wts


def _chunks_for(load):
    """Split a token count into moving-dim chunks: all but the last are
    multiples of 128 in [256, 512]; keep the ragged tail >= 240 when
    possible (short moving dims go LDWEIGHTS-bound)."""
    C = load
    n = max(1, -(-C // 512))
    chunks = []
    rem = C
    for i in range(n - 1):
        c = min(512, -(-rem // ((n - i) * 128)) * 128)
        chunks.append(c)
        rem -= c
    while n > 1 and rem < 240 and chunks:
        for i in range(len(chunks)):
            if rem >= 240:
                break
            if chunks[i] > 256:
                chunks[i] -= 128
                rem += 128
        else:
            break
    chunks.append(rem)
    assert sum(chunks) == C and all(c > 0 for c in chunks)
    return tuple(chunks)


def _plan(loads):
    """Per-expert chunk tuples + 128-aligned xT column offsets."""
    chunks_e, offs = [], []
    off = 0
    for l in loads:
        chunks_e.append(_chunks_for(l))
        offs.append(off)
        off += -(-l // 128) * 128
    return tuple(chunks_e), tuple(offs), off


# ------------------------------------------------------------- device IR ----
def _build(plan):
    """Per-core Bacc graph. plan = (chunks_e, offs, XWT)."""
    import concourse.bacc as bacc
    import concourse.bass as bass
    import concourse.mybir as mybir
    import concourse.tile as tile

    chunks_e, offs, XWT = plan
    XW_e = [-(-sum(ch) // 128) * 128 for ch in chunks_e]

    bf16 = mybir.dt.bfloat16
    f32 = mybir.dt.float32

    nc = bacc.Bacc("TRN2", target_bir_lowering=False, debug=False,
                   num_devices=N_CORES)

    xT_d = nc.dram_tensor("xT", [HIDDEN, XWT], bf16, kind="ExternalInput")
    # w1s/w3s host-pre-tiled per expert as [e, p, dc, FS]; w2s as
    # [e, p, ft, HIDDEN] so every DMA line is fully contiguous
    w1_d = nc.dram_tensor("w1s", [NUM_EXPERTS, 128, DC, FS], bf16,
                          kind="ExternalInput")
    w3_d = nc.dram_tensor("w3s", [NUM_EXPERTS, 128, DC, FS], bf16,
                          kind="ExternalInput")
    w2_d = nc.dram_tensor("w2s", [NUM_EXPERTS, 128, FT, HIDDEN], bf16,
                          kind="ExternalInput")
    y_d = nc.dram_tensor("y", [HIDDEN, XWT], bf16, kind="ExternalOutput")

    xT_v = xT_d.ap().rearrange("(dc p) c -> p dc c", p=128)
    y_v = y_d.ap().rearrange("(dt p) c -> dt p c", p=128)

    with tile.TileContext(nc) as tc:
        with (
            tc.tile_pool(name="xe", bufs=2) as xep,
            tc.tile_pool(name="w13", bufs=2) as w13,
            tc.tile_pool(name="w2p", bufs=2) as w2p,
            tc.tile_pool(name="hp", bufs=2) as hp,
            tc.tile_pool(name="sil", bufs=4) as silp,
            tc.tile_pool(name="yo", bufs=4) as yop,
            tc.tile_pool(name="ps", bufs=2, space=bass.MemorySpace.PSUM) as ps,
            tc.tile_pool(name="yps", bufs=4, space=bass.MemorySpace.PSUM) as yps,
        ):
            # HAM pre-warm: keep the PE's activity monitor busy during the
            # DMA-bound first ~10us so real matmuls start at full clock.
            warm_sb = silp.tile([128, 128], bf16, tag="warm_in", bufs=1)
            nc.gpsimd.memset(warm_sb[:], 0.0)
            warm_ps = ps.tile([128, 128], f32, tag="ph1", name="warm_ps")
            # sized to span kernel entry (~6.5us) -> first weights landed
            # (~14.5us): ends just as real work becomes ready, keeping the
            # clock ramped with no PE gap.
            N_WARM = 95
            for i in range(N_WARM):
                nc.tensor.matmul(warm_ps[:], warm_sb[:], warm_sb[:],
                                 start=(i == 0), stop=(i == N_WARM - 1))

            xe_tiles = {}
            w13_tiles = {}
            w2_tiles = {}

            def load_w13(e):
                w1_sb = w13.tile([128, DC, FS], bf16, tag="w1", name="w1_sb")
                w3_sb = w13.tile([128, DC, FS], bf16, tag="w3", name="w3_sb")
                nc.sync.dma_start(w1_sb[:], w1_d.ap()[e])
                nc.sync.dma_start(w3_sb[:], w3_d.ap()[e])
                w13_tiles[e] = ([(w1_sb, 0)], [(w3_sb, 0)])

            def load_xe(e, col0=0):
                if e not in xe_tiles:
                    xe_tiles[e] = xep.tile([128, DC, XW_e[e]], bf16,
                                           tag="xT", name="xe_sb")
                nc.sync.dma_start(
                    xe_tiles[e][:, :, col0:XW_e[e]],
                    xT_v[:, :, offs[e] + col0:offs[e] + XW_e[e]],
                )

            def load_w2(e):
                w2_sb = w2p.tile([128, FT, HIDDEN], bf16, tag="w2",
                                 name="w2_sb")
                nc.sync.dma_start(w2_sb[:], w2_d.ap()[e])
                w2_tiles[e] = w2_sb

            def _wslice(parts, dc):
                for tile_, base in parts:
                    if base <= dc < base + tile_.shape[1]:
                        return tile_[:, dc - base, :]
                raise AssertionError(dc)

            # startup: the first f-tile chain needs w1 dc-tiles + xT chunk0;
            # split w1/w3 into dc-halves and put the critical ones first.
            c0 = chunks_e[0][0]
            w1a = w13.tile([128, 4, FS], bf16, tag="w1a", bufs=1)
            w3a = w13.tile([128, 4, FS], bf16, tag="w3a", bufs=1)
            w1b = w13.tile([128, 4, FS], bf16, tag="w1b", bufs=1)
            w3b = w13.tile([128, 4, FS], bf16, tag="w3b", bufs=1)
            xe_tiles[0] = xep.tile([128, DC, XW_e[0]], bf16, tag="xT",
                                   name="xe_sb0")
            nc.sync.dma_start(w1a[:], w1_d.ap()[0][:, 0:4, :])
            nc.sync.dma_start(xe_tiles[0][:, :, 0:c0], xT_v[:, :, 0:c0])
            nc.sync.dma_start(w1b[:], w1_d.ap()[0][:, 4:DC, :])
            nc.sync.dma_start(w3a[:], w3_d.ap()[0][:, 0:4, :])
            nc.sync.dma_start(w3b[:], w3_d.ap()[0][:, 4:DC, :])
            w13_tiles[0] = ([(w1a, 0), (w1b, 4)], [(w3a, 0), (w3b, 4)])

            for e in range(NUM_EXPERTS):
                w1_parts, w3_parts = w13_tiles[e]
                t0 = 0
                for ci, chunk in enumerate(chunks_e[e]):
                    xe = xe_tiles[e]
                    hT = hp.tile([128, FT, chunk], bf16, tag="hT")
                    # ---- phase A ----
                    for ft in range(FT):
                        # prefetches ride behind the first chunk's compute
                        if ci == 0:
                            if e == 0:
                                if ft == 1:
                                    load_xe(0, col0=c0)
                                    load_w2(0)
                                elif ft == 2:
                                    load_w13(1)
                                elif ft == 3:
                                    load_xe(1)
                                    load_w2(1)
                            elif e + 1 < NUM_EXPERTS:
                                if ft == 2:
                                    load_w13(e + 1)
                                elif ft == 3:
                                    load_xe(e + 1)
                                    load_w2(e + 1)
                        ph1 = ps.tile([128, chunk], f32, tag="ph1")
                        ph3 = ps.tile([128, chunk], f32, tag="ph3")
                        for dc in range(DC):
                            nc.tensor.matmul(
                                ph1[:],
                                _wslice(w1_parts, dc)[
                                    :, ft * 128:(ft + 1) * 128],
                                xe[:, dc, t0:t0 + chunk],
                                start=(dc == 0), stop=(dc == DC - 1),
                            )
                        for dc in range(DC):
                            nc.tensor.matmul(
                                ph3[:],
                                _wslice(w3_parts, dc)[
                                    :, ft * 128:(ft + 1) * 128],
                                xe[:, dc, t0:t0 + chunk],
                                start=(dc == 0), stop=(dc == DC - 1),
                            )
                        sil = silp.tile([128, chunk], bf16, tag="sil")
                        nc.scalar.activation(
                            sil[:], ph1[:], mybir.ActivationFunctionType.Silu
                        )
                        nc.vector.tensor_mul(hT[:, ft, :], sil[:], ph3[:])

                    # ---- phase B: yT[d, t] partial, unscaled ----
                    w2_sb = w2_tiles[e]
                    for dt in range(DT):
                        yp = yps.tile([128, chunk], f32, tag="yp")
                        for ft in range(FT):
                            nc.tensor.matmul(
                                yp[:],
                                w2_sb[:, ft, dt * 128:(dt + 1) * 128],
                                hT[:, ft, :],
                                start=(ft == 0), stop=(ft == FT - 1),
                            )
                        ysb = yop.tile([128, chunk], bf16, tag="ysb")
                        # alternate copies between ScalarE and DVE so
                        # neither engine becomes the bottleneck
                        if dt % 2 == 0:
                            nc.scalar.copy(ysb[:], yp[:])
                        else:
                            nc.vector.tensor_copy(ysb[:], yp[:])
                        nc.sync.dma_start(
                            y_v[dt][:, offs[e] + t0:offs[e] + t0 + chunk],
                            ysb[:],
                        )
                    t0 += chunk
    nc.compile()
    return nc


def _get_nc(plan):
    if plan not in _nc_cache:
        _nc_cache[plan] = _build(plan)
    return _nc_cache[plan]


# ---------------------------------------------------------------- kernel ----
def kernel(hidden_states, gate_w, gate_b, w1, w3, w2, _trace=False):
    from concourse.bass_utils import run_bass_kernel_spmd

    B, S, D = hidden_states.shape
    T = B * S
    x = np.asarray(hidden_states, np.float32).reshape(T, D)
    idxs, wts = _route(x, np.asarray(gate_w, np.float32),
                       np.asarray(gate_b, np.float32))
    loads = [len(i) for i in idxs]
    plan = _plan(loads)
    chunks_e, offs, XWT = plan
    nc = _get_nc(plan)

    # shared xT: every expert's tokens in its 128-aligned column range
    xT = np.zeros((D, XWT), _BF16)
    for e in range(NUM_EXPERTS):
        xT[:, offs[e]:offs[e] + loads[e]] = x[idxs[e]].T.astype(_BF16)

    # weights pre-tiled for all cores in one reshape/transpose:
    # w1/w3 [E, D, F] -> [core, E, 128, dc, FS]
    w1 = np.asarray(w1, np.float32)
    w3 = np.asarray(w3, np.float32)
    w2 = np.asarray(w2, np.float32)
    w1t = np.ascontiguousarray(
        w1.reshape(NUM_EXPERTS, DC, 128, N_CORES, FS)
        .transpose(3, 0, 2, 1, 4)).astype(_BF16)
    w3t = np.ascontiguousarray(
        w3.reshape(NUM_EXPERTS, DC, 128, N_CORES, FS)
        .transpose(3, 0, 2, 1, 4)).astype(_BF16)
    # w2 [E, F, D] -> [core, E, 128, ft, D]
    w2t = np.ascontiguousarray(
        w2.reshape(NUM_EXPERTS, N_CORES, FT, 128, HIDDEN)
        .transpose(1, 0, 3, 2, 4)).astype(_BF16)

    in_maps = [{
        "xT": xT,
        "w1s": w1t[c],
        "w3s": w3t[c],
        "w2s": w2t[c],
    } for c in range(N_CORES)]

    res = run_bass_kernel_spmd(nc, in_maps, core_ids=list(range(N_CORES)),
                               trace=_trace)

    yT = res.results[0]["y"].astype(np.float32)
    for c in range(1, N_CORES):
        yT += res.results[c]["y"].astype(np.float32)
    out = np.zeros((T, D), np.float32)
    for e in range(NUM_EXPERTS):
        tok, wt = idxs[e], wts[e]
        seg = yT[:, offs[e]:offs[e] + loads[e]].T  # [load, D]
        out[tok] += wt[:, None] * seg
    out = out.reshape(B, S, D)
    if _trace:
        return out, res
    return out
